# revision 37
# baseline (speedup 1.0000x reference)
"""B-spline (clamped) surface evaluation on 8 Trainium2 cores.

Math: out[u, v, :] = sum_{a,b} Bu[u,a] * Bv[v,b] * P[su[u]-p+a, sv[v]-p+b, :]

Host precomputes the tiny Cox-de-Boor basis, scatters it into dense matrices
Au [Nu, 64], Av [Nv, 64], and folds the small control-point contraction
T[u, j, d] = sum_i Au[u, i] P[i, j, d] (25M MACs, fp64 on host). The device
then does the dominant contraction (768M MACs):

  S[u, v, d] = sum_j T[u, j, d] * Av[v, j]       (TensorEngine matmuls)

The rel-err gate is 2e-2, so everything device-side runs in plain bf16
(~3e-3 total): no hi/lo split, and the output is written to HBM as bf16
(24 MB total instead of 48 MB) with the host casting back to fp32.

The K=64 contraction is zero-padded to K=128 (rows 64-127 = 0) so the
matmuls use the standard full-array config; matmul cycles scale with the
streamed column count, not K, so the padding is free.  Only the real 64
rows are DMA'd; the zero halves are memset on-device by the otherwise-idle
DVE/ACT engines during the startup window.

Latency choreography (the kernel is dependency-bound, not
throughput-bound): the NEFF start barrier gates everything until ~6.5 us,
and each input DMA pays gen + transfer + ~1.5-2 us completion receipt.  So
inputs are split into 5 independent tiles, ordered by first use, across
two parallel rings (HWDGE sync: tta, avt1, avt2; SWDGE: avt3, ttb), and 6
dummy warmup matmuls on a zeroed tile keep the PE busy from the barrier
until real data lands -- both bridging the HAM clock-gate window (PE runs
1.2 GHz until ~3.4 us of sustained activity) and wasting no warm-up time.

Each (u-tile, d) group runs LDWEIGHTS once then streams 4 x N<=512 matmuls
into two 2-bank PSUM tiles; DVE and ACT alternate evacuating them with the
fp32->bf16 cast fused into wide [128, ~1000] copies (fp32 PSUM reads run at
1 elem/cycle/lane, so wide copies amortize the ~120-170 cycle fixed cost).
The group's [128, 2001] bf16 output region (512 KB) is flushed to HBM as
soon as its two copies land, round-robined over 4 SWDGE queues, so the
output DMA (~8.3 us/core at the ~358 GB/s HBM roofline) overlaps compute;
the final group flushes in two halves to shorten the completion tail.
Flushes always cover all 128 partitions: partial-partition DMAs were
measured to unbalance the SDMA engine split ~3x.

Sharding: data-parallel over u. Each core computes a [251, 2001, 3] slab,
padded to 2x128 u-rows on device; the host drops the padding and
interleaves d.
"""

import numpy as np

N_CTRL = 64
N_EVAL = 2001
N_CORES = 8
NU_SHARD = 251   # ceil(2001 / 8); 8 * 251 = 2008 (last 7 rows are zero padding)
NU_PAD = 256     # per-core u padded to 2 full 128-wide PE column tiles
V_TILE = 512
V_HALF = 1024    # psum/copy/flush half-split of the v axis

_CACHE = {}


def _clamped_knots(p, n_ctrl, dtype=np.float64):
    n_internal = n_ctrl - p - 1
    internal = np.linspace(0.0, 1.0, n_internal + 2, dtype=dtype)[1:-1]
    return np.concatenate(
        [np.zeros(p + 1, dtype), internal, np.ones(p + 1, dtype)]
    )


def _dense_basis(params, p, n_ctrl):
    """Dense basis matrix A [len(params), n_ctrl], float64, with
    A[k, span-p+a] = B[k, a] (Cox-de-Boor, NURBS book A2.2)."""
    knots = _clamped_knots(p, n_ctrl)
    u = np.asarray(params, np.float64)
    spans = np.clip(np.searchsorted(knots, u, side="right") - 1, p, n_ctrl - 1)
    Ns = [np.ones_like(u)]
    left = {}
    right = {}
    for j in range(1, p + 1):
        left[j] = u - knots[spans + 1 - j]
        right[j] = knots[spans + j] - u
        saved = np.zeros_like(u)
        new = []
        for r in range(j):
            temp = Ns[r] / (right[r + 1] + left[j - r])
            new.append(saved + right[r + 1] * temp)
            saved = left[j - r] * temp
        new.append(saved)
        Ns = new
    B = np.stack(Ns, axis=-1)  # [N, p+1]
    A = np.zeros((len(u), n_ctrl), np.float64)
    rows = np.arange(len(u))[:, None]
    cols = spans[:, None] - p + np.arange(p + 1)[None, :]
    A[rows, cols] = B
    return A


def _pad_k128(a_bf16):
    """[64, N] bf16 -> [128, N] with zeros in partitions 64-127."""
    return np.ascontiguousarray(
        np.concatenate([a_bf16, np.zeros_like(a_bf16)], axis=0)
    )


# input tiles: name -> columns.  head = first weight group (d=0, g=0) plus
# the first Av.T v-tile fused into one tile so a single DMA receipt gates
# both the first LDWEIGHTS and the first matmul; ttb = the other five
# weight groups; avt2/3 = remaining Av.T columns in first-use order.
IN_COLS = {
    "head": 128 + V_TILE,
    "ttb": 3 * NU_PAD - 128,
    "avt2": V_TILE,
    "avt3": N_EVAL - V_HALF,
}


def _build_device():
    if "nc" in _CACHE:
        return _CACHE["nc"]

    import concourse.mybir as mybir
    import concourse.tile as tile
    from concourse import bacc

    f32 = mybir.dt.float32
    bf16 = mybir.dt.bfloat16
    nc = bacc.Bacc(
        "TRN2", target_bir_lowering=False, debug=False, num_devices=N_CORES,
        num_swdge_queues=4,
    )
    # inputs ship as [128, cols] with host-side zeros in rows 64-127: a
    # 64-partition DMA would engage only the 8 even SDMA engines (engine <->
    # partition mapping), measured slower and jitterier than moving 2x the
    # bytes through all 16
    ins = {
        name: nc.dram_tensor(name, [128, cols], bf16,
                             kind="ExternalInput").ap()
        for name, cols in IN_COLS.items()
    }
    # out col = g*6003 + d*2001 + v for u-tile g in {0, 1}
    out_h = nc.dram_tensor(
        "out", [128, 2 * 3 * N_EVAL], bf16, kind="ExternalOutput"
    ).ap()

    # (v0, width, avt tile, col offset within it) in stream order
    VT = [
        (0, V_TILE, "head", 128),
        (V_TILE, V_TILE, "avt2", 0),
        (V_HALF, V_TILE, "avt3", 0),
        (V_HALF + V_TILE, N_EVAL - V_HALF - V_TILE, "avt3", V_TILE),
    ]

    with tile.TileContext(nc) as tc:
        with (
            tc.tile_pool(name="consts", bufs=1) as consts,
            tc.tile_pool(name="ps", bufs=4, space="PSUM") as psp,
            tc.tile_pool(name="obuf", bufs=1) as obuf,
        ):
            sb = {
                name: consts.tile([128, cols], bf16, tag=name, name=name)
                for name, cols in IN_COLS.items()
            }
            # head/avt2/avt3 ride the sync HWDGE ring FIFO, first-use first:
            # serialized packets mean the head's completion semaphore can't
            # straggle behind other queues in the SDMA round-robin (the
            # interleaved 4-queue layout measured 10.3-11.6 us receipt
            # jitter on the slowest core).  ttb overlaps on a SWDGE queue.
            for name in ("head", "avt2", "avt3"):
                nc.sync.dma_start(out=sb[name], in_=ins[name])
            dma = nc.gpsimd.dma_start(out=sb["ttb"], in_=ins["ttb"])
            dma.ins.queue = "qPoolDynamic1"
            warm = consts.tile([128, V_TILE], bf16, tag="warm", name="warm")
            nc.vector.memset(warm, 0.0)

            # PE warmup: bridge barrier-to-data-landing so the HAM clock
            # gate sees sustained activity (cold PE = 1.2 GHz for ~3.4 us).
            # N=512 dummies measured the most reliable un-throttle (N=256
            # ones ran the same span but HAM fired ~4 us later).  Overshoot
            # the typical input receipt: an idle gap before the real stream
            # delays the un-throttle by a whole extra ~3.4 us window on the
            # slowest core, which is what the max-core exec time pays.
            for _ in range(10):
                wps = psp.tile([128, V_HALF], f32, tag="ps")
                nc.tensor.matmul(
                    wps[:, :V_TILE], warm[:, :128], warm,
                    start=True, stop=True,
                )

            ob = {
                g: obuf.tile([128, 3 * N_EVAL], bf16, tag=f"ob{g}",
                             name=f"ob{g}")
                for g in range(2)
            }

            prev_mm = None
            n_out = 0
            gi = 0
            for d in range(3):
                for g in range(2):
                    col = d * NU_PAD + g * 128
                    if col == 0:
                        w = sb["head"][:, 0:128]
                    else:
                        w = sb["ttb"][:, col - 128:col]
                    ldw = nc.tensor.ldweights(w)
                    if prev_mm is not None:
                        tile.add_dep_helper(
                            ldw.ins, prev_mm.ins, sync=False,
                            reason="weight group order",
                        )
                    # the first group flushes per half-region so the output
                    # DMA window (the line-rate-dense ~9 us that paces the
                    # kernel) opens as early as possible; the final group
                    # flushes per half to shorten the end-of-kernel
                    # copy+transfer+receipt tail
                    split = gi in (0, 5)
                    obase = g * 3 * N_EVAL + d * N_EVAL
                    for hi, (h0, hw) in enumerate(((0, V_HALF),
                                                   (V_HALF, N_EVAL - V_HALF))):
                        ps = psp.tile([128, V_HALF], f32, tag="ps")
                        for v0, vw, av, c0 in VT[2 * hi:2 * hi + 2]:
                            mm = nc.tensor.matmul(
                                ps[:, v0 - h0:v0 - h0 + vw], w,
                                sb[av][:, c0:c0 + vw],
                                start=True, stop=True,
                            )
                            mm.ins.ldweights = False
                            tile.add_dep_helper(
                                mm.ins, ldw.ins, sync=False,
                                reason="matmul after its ldweights",
                            )
                            prev_mm = mm
                        osl = slice(d * N_EVAL + h0, d * N_EVAL + h0 + hw)
                        if split:
                            # first/last group: both engines evacuate the
                            # half concurrently (one v-tile each) so the
                            # flush fires ~0.5 us after the half's last
                            # matmul instead of ~1.1
                            nc.vector.tensor_copy(
                                ob[g][:, slice(d * N_EVAL + h0,
                                               d * N_EVAL + h0 + V_TILE)],
                                ps[:, :V_TILE])
                            nc.scalar.copy(
                                ob[g][:, slice(d * N_EVAL + h0 + V_TILE,
                                               d * N_EVAL + h0 + hw)],
                                ps[:, V_TILE:hw])
                        # one wide copy per half; engines alternate and
                        # disjoint ranges run concurrently
                        elif (hi == 0) == (gi % 2 == 0):
                            nc.vector.tensor_copy(ob[g][:, osl], ps[:, :hw])
                        else:
                            nc.scalar.copy(ob[g][:, osl], ps[:, :hw])
                        if split:
                            dma = nc.gpsimd.dma_start(
                                out=out_h[:, obase + h0:obase + h0 + hw],
                                in_=ob[g][:, osl],
                            )
                            dma.ins.queue = f"qPoolDynamic{n_out % 4 or ''}"
                            n_out += 1
                    if not split:
                        # flush this (u-tile, d) region once its copies land
                        osl = slice(d * N_EVAL, (d + 1) * N_EVAL)
                        dma = nc.gpsimd.dma_start(
                            out=out_h[:, obase:obase + N_EVAL],
                            in_=ob[g][:, osl],
                        )
                        dma.ins.queue = f"qPoolDynamic{n_out % 4 or ''}"
                        n_out += 1
                    gi += 1
    nc.compile()
    _CACHE["nc"] = nc
    return nc


def kernel(control_points, params_u, params_v, degree):
    import ml_dtypes
    from concourse.bass_utils import run_bass_kernel_spmd

    p = int(np.asarray(degree))
    cp = np.asarray(control_points, np.float32)
    pu = np.asarray(params_u, np.float32)
    pv = np.asarray(params_v, np.float32)
    assert cp.shape == (N_CTRL, N_CTRL, 3), cp.shape
    assert pu.shape == (N_EVAL,) and pv.shape == (N_EVAL,), (pu.shape, pv.shape)

    Au = np.zeros((N_CORES * NU_SHARD, N_CTRL), np.float64)
    Au[:N_EVAL] = _dense_basis(pu, p, N_CTRL)
    Av = _dense_basis(pv, p, N_CTRL)

    # host stage 1 (0.3% of the FLOPs): T[j, d, u] = sum_i P[i,j,d] Au[u,i]
    T = (cp.astype(np.float64).transpose(1, 2, 0).reshape(3 * N_CTRL, N_CTRL)
         @ Au.T).reshape(N_CTRL, 3, N_CORES * NU_SHARD)

    avt = Av.T.astype(np.float32).astype(ml_dtypes.bfloat16)
    avs = {
        "avt2": _pad_k128(np.ascontiguousarray(avt[:, V_TILE:V_HALF])),
        "avt3": _pad_k128(np.ascontiguousarray(avt[:, V_HALF:])),
    }

    nc = _build_device()
    in_maps = []
    for c in range(N_CORES):
        ttc = np.zeros((N_CTRL, 3, NU_PAD), np.float32)
        ttc[:, :, :NU_SHARD] = T[:, :, c * NU_SHARD:(c + 1) * NU_SHARD]
        tt = ttc.reshape(N_CTRL, 3 * NU_PAD).astype(ml_dtypes.bfloat16)
        in_maps.append({
            "head": _pad_k128(np.ascontiguousarray(
                np.concatenate([tt[:, :128], avt[:, :V_TILE]], axis=1)
            )),
            "ttb": _pad_k128(np.ascontiguousarray(tt[:, 128:])),
            **avs,
        })

    res = run_bass_kernel_spmd(
        nc,
        in_maps,
        core_ids=list(range(N_CORES)),
        trace=_CACHE.get("trace", False),
        **_CACHE.get("run_kwargs", {}),
    )
    _CACHE["last_result"] = res
    # out col = g*6003 + d*2001 + v; u-tile g=1 holds rows 128..250
    full = np.empty((N_CORES * NU_SHARD, 3, N_EVAL), np.float32)
    for c, r in enumerate(res.results):
        o = np.asarray(r["out"]).astype(np.float32)
        o = o.reshape(128, 2, 3, N_EVAL)  # cols are [g][d][v] row-major
        full[c * NU_SHARD:c * NU_SHARD + 128] = o[:, 0]
        full[c * NU_SHARD + 128:(c + 1) * NU_SHARD] = o[:NU_SHARD - 128, 1]
    return np.ascontiguousarray(full[:N_EVAL].transpose(0, 2, 1))


# revision 38
# speedup vs baseline: 1.0860x; 1.0860x over previous
"""B-spline (clamped) surface evaluation on 8 Trainium2 cores.

Math: out[u, v, :] = sum_{a,b} Bu[u,a] * Bv[v,b] * P[su[u]-p+a, sv[v]-p+b, :]

Host precomputes the tiny Cox-de-Boor basis, scatters it into dense matrices
Au [Nu, 64], Av [Nv, 64], and folds the small control-point contraction
T[u, j, d] = sum_i Au[u, i] P[i, j, d] (25M MACs, fp64 on host). The device
then does the dominant contraction (768M MACs):

  S[u, v, d] = sum_j T[u, j, d] * Av[v, j]       (TensorEngine matmuls)

The rel-err gate is 2e-2, so everything device-side runs in plain bf16
(~3e-3 total): no hi/lo split, and the output is written to HBM as bf16
(24 MB total instead of 48 MB) with the host casting back to fp32.

The K=64 contraction is zero-padded to K=128 (rows 64-127 = 0) so the
matmuls use the standard full-array config; matmul cycles scale with the
streamed column count, not K, so the padding is free.  Only the real 64
rows are DMA'd; the zero halves are memset on-device by the otherwise-idle
DVE/ACT engines during the startup window.

Latency choreography (the kernel is dependency-bound, not
throughput-bound): the NEFF start barrier gates everything until ~6.5 us,
and each input DMA pays gen + transfer + ~1.5-2 us completion receipt.  So
inputs are split into 5 independent tiles, ordered by first use, across
two parallel rings (HWDGE sync: tta, avt1, avt2; SWDGE: avt3, ttb), and 6
dummy warmup matmuls on a zeroed tile keep the PE busy from the barrier
until real data lands -- both bridging the HAM clock-gate window (PE runs
1.2 GHz until ~3.4 us of sustained activity) and wasting no warm-up time.

Each (u-tile, d) group runs LDWEIGHTS once then streams 4 x N<=512 matmuls
into two 2-bank PSUM tiles; DVE and ACT alternate evacuating them with the
fp32->bf16 cast fused into wide [128, ~1000] copies (fp32 PSUM reads run at
1 elem/cycle/lane, so wide copies amortize the ~120-170 cycle fixed cost).
The group's [128, 2001] bf16 output region (512 KB) is flushed to HBM as
soon as its two copies land, round-robined over 4 SWDGE queues, so the
output DMA (~8.3 us/core at the ~358 GB/s HBM roofline) overlaps compute;
the final group flushes in two halves to shorten the completion tail.
Flushes always cover all 128 partitions: partial-partition DMAs were
measured to unbalance the SDMA engine split ~3x.

Sharding: data-parallel over u. Each core computes a [251, 2001, 3] slab,
padded to 2x128 u-rows on device; the host drops the padding and
interleaves d.
"""

import numpy as np

N_CTRL = 64
N_EVAL = 2001
N_CORES = 8
NU_SHARD = 251   # ceil(2001 / 8); 8 * 251 = 2008 (last 7 rows are zero padding)
NU_PAD = 256     # per-core u padded to 2 full 128-wide PE column tiles
V_TILE = 512
V_HALF = 1024    # psum/copy/flush half-split of the v axis

_CACHE = {}


def _clamped_knots(p, n_ctrl, dtype=np.float64):
    n_internal = n_ctrl - p - 1
    internal = np.linspace(0.0, 1.0, n_internal + 2, dtype=dtype)[1:-1]
    return np.concatenate(
        [np.zeros(p + 1, dtype), internal, np.ones(p + 1, dtype)]
    )


def _dense_basis(params, p, n_ctrl):
    """Dense basis matrix A [len(params), n_ctrl], float64, with
    A[k, span-p+a] = B[k, a] (Cox-de-Boor, NURBS book A2.2)."""
    knots = _clamped_knots(p, n_ctrl)
    u = np.asarray(params, np.float64)
    spans = np.clip(np.searchsorted(knots, u, side="right") - 1, p, n_ctrl - 1)
    Ns = [np.ones_like(u)]
    left = {}
    right = {}
    for j in range(1, p + 1):
        left[j] = u - knots[spans + 1 - j]
        right[j] = knots[spans + j] - u
        saved = np.zeros_like(u)
        new = []
        for r in range(j):
            temp = Ns[r] / (right[r + 1] + left[j - r])
            new.append(saved + right[r + 1] * temp)
            saved = left[j - r] * temp
        new.append(saved)
        Ns = new
    B = np.stack(Ns, axis=-1)  # [N, p+1]
    A = np.zeros((len(u), n_ctrl), np.float64)
    rows = np.arange(len(u))[:, None]
    cols = spans[:, None] - p + np.arange(p + 1)[None, :]
    A[rows, cols] = B
    return A


def _pad_k128(a_bf16):
    """[64, N] bf16 -> [128, N] with zeros in partitions 64-127."""
    return np.ascontiguousarray(
        np.concatenate([a_bf16, np.zeros_like(a_bf16)], axis=0)
    )


# input tiles: name -> columns.  head = first weight group (d=0, g=0) plus
# the first Av.T v-tile fused into one tile so a single DMA receipt gates
# both the first LDWEIGHTS and the first matmul; ttb = the other five
# weight groups; avt2/3 = remaining Av.T columns in first-use order.
IN_COLS = {
    "head": 128 + V_TILE,
    "ttb": 3 * NU_PAD - 128,
    "avt2": V_TILE,
    "avt3": N_EVAL - V_HALF,
}


def _build_device():
    if "nc" in _CACHE:
        return _CACHE["nc"]

    import concourse.mybir as mybir
    import concourse.tile as tile
    from concourse import bacc

    f32 = mybir.dt.float32
    bf16 = mybir.dt.bfloat16
    nc = bacc.Bacc(
        "TRN2", target_bir_lowering=False, debug=False, num_devices=N_CORES,
        num_swdge_queues=4,
    )
    # inputs ship as [128, cols] with host-side zeros in rows 64-127: a
    # 64-partition DMA would engage only the 8 even SDMA engines (engine <->
    # partition mapping), measured slower and jitterier than moving 2x the
    # bytes through all 16
    ins = {
        name: nc.dram_tensor(name, [128, cols], bf16,
                             kind="ExternalInput").ap()
        for name, cols in IN_COLS.items()
    }
    # out col = g*6003 + d*2001 + v for u-tile g in {0, 1}
    out_h = nc.dram_tensor(
        "out", [128, 2 * 3 * N_EVAL], bf16, kind="ExternalOutput"
    ).ap()

    # (v0, width, avt tile, col offset within it) in stream order
    VT = [
        (0, V_TILE, "head", 128),
        (V_TILE, V_TILE, "avt2", 0),
        (V_HALF, V_TILE, "avt3", 0),
        (V_HALF + V_TILE, N_EVAL - V_HALF - V_TILE, "avt3", V_TILE),
    ]

    with tile.TileContext(nc) as tc:
        with (
            tc.tile_pool(name="consts", bufs=1) as consts,
            tc.tile_pool(name="ps", bufs=4, space="PSUM") as psp,
            tc.tile_pool(name="obuf", bufs=1) as obuf,
        ):
            sb = {
                name: consts.tile([128, cols], bf16, tag=name, name=name)
                for name, cols in IN_COLS.items()
            }
            # head/avt2/avt3 ride the sync HWDGE ring FIFO, first-use first:
            # serialized packets mean the head's completion semaphore can't
            # straggle behind other queues in the SDMA round-robin (the
            # interleaved 4-queue layout measured 10.3-11.6 us receipt
            # jitter on the slowest core).  ttb overlaps on a SWDGE queue.
            for name in ("head", "avt2", "avt3"):
                nc.sync.dma_start(out=sb[name], in_=ins[name])
            dma = nc.gpsimd.dma_start(out=sb["ttb"], in_=ins["ttb"])
            dma.ins.queue = "qPoolDynamic1"
            warm = consts.tile([128, V_TILE], bf16, tag="warm", name="warm")
            nc.vector.memset(warm, 0.0)

            # PE warmup: bridge barrier-to-data-landing so the HAM clock
            # gate sees sustained activity (cold PE = 1.2 GHz for ~3.4 us).
            # N=512 dummies measured the most reliable un-throttle (N=256
            # ones ran the same span but HAM fired ~4 us later).  Overshoot
            # the typical input receipt: an idle gap before the real stream
            # delays the un-throttle by a whole extra ~3.4 us window on the
            # slowest core, which is what the max-core exec time pays.
            for _ in range(10):
                wps = psp.tile([128, V_HALF], f32, tag="ps")
                nc.tensor.matmul(
                    wps[:, :V_TILE], warm[:, :128], warm,
                    start=True, stop=True,
                )

            ob = {
                g: obuf.tile([128, 3 * N_EVAL], bf16, tag=f"ob{g}",
                             name=f"ob{g}")
                for g in range(2)
            }

            prev_mm = None
            n_out = 0
            gi = 0
            for d in range(3):
                for g in range(2):
                    col = d * NU_PAD + g * 128
                    if col == 0:
                        w = sb["head"][:, 0:128]
                    else:
                        w = sb["ttb"][:, col - 128:col]
                    ldw = nc.tensor.ldweights(w)
                    if prev_mm is not None:
                        tile.add_dep_helper(
                            ldw.ins, prev_mm.ins, sync=False,
                            reason="weight group order",
                        )
                    # the first group flushes per half-region so the output
                    # DMA window (the line-rate-dense ~9 us that paces the
                    # kernel) opens as early as possible; the final group
                    # flushes per half to shorten the end-of-kernel
                    # copy+transfer+receipt tail
                    split = gi in (0, 5)
                    obase = g * 3 * N_EVAL + d * N_EVAL
                    for hi, (h0, hw) in enumerate(((0, V_HALF),
                                                   (V_HALF, N_EVAL - V_HALF))):
                        ps = psp.tile([128, V_HALF], f32, tag="ps")
                        for v0, vw, av, c0 in VT[2 * hi:2 * hi + 2]:
                            mm = nc.tensor.matmul(
                                ps[:, v0 - h0:v0 - h0 + vw], w,
                                sb[av][:, c0:c0 + vw],
                                start=True, stop=True,
                            )
                            mm.ins.ldweights = False
                            tile.add_dep_helper(
                                mm.ins, ldw.ins, sync=False,
                                reason="matmul after its ldweights",
                            )
                            prev_mm = mm
                        osl = slice(d * N_EVAL + h0, d * N_EVAL + h0 + hw)
                        # one wide copy per half; engines alternate and
                        # disjoint ranges run concurrently (per-v-tile
                        # two-engine copies were tried three times and
                        # always measured slower end-to-end)
                        if (hi == 0) == (gi % 2 == 0):
                            nc.vector.tensor_copy(ob[g][:, osl], ps[:, :hw])
                        else:
                            nc.scalar.copy(ob[g][:, osl], ps[:, :hw])
                        if split:
                            dma = nc.gpsimd.dma_start(
                                out=out_h[:, obase + h0:obase + h0 + hw],
                                in_=ob[g][:, osl],
                            )
                            dma.ins.queue = f"qPoolDynamic{n_out % 4 or ''}"
                            n_out += 1
                    if not split:
                        # flush this (u-tile, d) region once its copies land
                        osl = slice(d * N_EVAL, (d + 1) * N_EVAL)
                        dma = nc.gpsimd.dma_start(
                            out=out_h[:, obase:obase + N_EVAL],
                            in_=ob[g][:, osl],
                        )
                        dma.ins.queue = f"qPoolDynamic{n_out % 4 or ''}"
                        n_out += 1
                    gi += 1
    nc.compile()
    _CACHE["nc"] = nc
    return nc


def kernel(control_points, params_u, params_v, degree):
    import ml_dtypes
    from concourse.bass_utils import run_bass_kernel_spmd

    p = int(np.asarray(degree))
    cp = np.asarray(control_points, np.float32)
    pu = np.asarray(params_u, np.float32)
    pv = np.asarray(params_v, np.float32)
    assert cp.shape == (N_CTRL, N_CTRL, 3), cp.shape
    assert pu.shape == (N_EVAL,) and pv.shape == (N_EVAL,), (pu.shape, pv.shape)

    Au = np.zeros((N_CORES * NU_SHARD, N_CTRL), np.float64)
    Au[:N_EVAL] = _dense_basis(pu, p, N_CTRL)
    Av = _dense_basis(pv, p, N_CTRL)

    # host stage 1 (0.3% of the FLOPs): T[j, d, u] = sum_i P[i,j,d] Au[u,i]
    T = (cp.astype(np.float64).transpose(1, 2, 0).reshape(3 * N_CTRL, N_CTRL)
         @ Au.T).reshape(N_CTRL, 3, N_CORES * NU_SHARD)

    avt = Av.T.astype(np.float32).astype(ml_dtypes.bfloat16)
    avs = {
        "avt2": _pad_k128(np.ascontiguousarray(avt[:, V_TILE:V_HALF])),
        "avt3": _pad_k128(np.ascontiguousarray(avt[:, V_HALF:])),
    }

    nc = _build_device()
    in_maps = []
    for c in range(N_CORES):
        ttc = np.zeros((N_CTRL, 3, NU_PAD), np.float32)
        ttc[:, :, :NU_SHARD] = T[:, :, c * NU_SHARD:(c + 1) * NU_SHARD]
        tt = ttc.reshape(N_CTRL, 3 * NU_PAD).astype(ml_dtypes.bfloat16)
        in_maps.append({
            "head": _pad_k128(np.ascontiguousarray(
                np.concatenate([tt[:, :128], avt[:, :V_TILE]], axis=1)
            )),
            "ttb": _pad_k128(np.ascontiguousarray(tt[:, 128:])),
            **avs,
        })

    res = run_bass_kernel_spmd(
        nc,
        in_maps,
        core_ids=list(range(N_CORES)),
        trace=_CACHE.get("trace", False),
        **_CACHE.get("run_kwargs", {}),
    )
    _CACHE["last_result"] = res
    # out col = g*6003 + d*2001 + v; u-tile g=1 holds rows 128..250
    full = np.empty((N_CORES * NU_SHARD, 3, N_EVAL), np.float32)
    for c, r in enumerate(res.results):
        o = np.asarray(r["out"]).astype(np.float32)
        o = o.reshape(128, 2, 3, N_EVAL)  # cols are [g][d][v] row-major
        full[c * NU_SHARD:c * NU_SHARD + 128] = o[:, 0]
        full[c * NU_SHARD + 128:(c + 1) * NU_SHARD] = o[:NU_SHARD - 128, 1]
    return np.ascontiguousarray(full[:N_EVAL].transpose(0, 2, 1))


# revision 39
# speedup vs baseline: 1.1405x; 1.0502x over previous
"""B-spline (clamped) surface evaluation on 8 Trainium2 cores.

Math: out[u, v, :] = sum_{a,b} Bu[u,a] * Bv[v,b] * P[su[u]-p+a, sv[v]-p+b, :]

Host precomputes the tiny Cox-de-Boor basis, scatters it into dense matrices
Au [Nu, 64], Av [Nv, 64], and folds the small control-point contraction
T[u, j, d] = sum_i Au[u, i] P[i, j, d] (25M MACs, fp64 on host). The device
then does the dominant contraction (768M MACs):

  S[u, v, d] = sum_j T[u, j, d] * Av[v, j]       (TensorEngine matmuls)

The rel-err gate is 2e-2, so everything device-side runs in plain bf16
(~3e-3 total): no hi/lo split, and the output is written to HBM as bf16
(24 MB total instead of 48 MB) with the host casting back to fp32.

The K=64 contraction is zero-padded to K=128 (rows 64-127 = 0) so the
matmuls use the standard full-array config; matmul cycles scale with the
streamed column count, not K, so the padding is free.  Inputs ship as
[128, cols]: a 64-partition DMA would engage only the 8 even SDMA engines
(engine <-> partition mapping), measured slower and jitterier than moving
2x the bytes through all 16.

Latency choreography (the kernel is dependency-bound, not
throughput-bound): the NEFF start barrier gates everything until ~6.5 us,
and each input DMA pays gen + transfer + ~1.5-2 us completion receipt.  So
inputs are split into 4 independent tiles ordered by first use -- "head"
fuses the first weight group with the first Av v-tile so one receipt gates
both the first LDWEIGHTS and first matmul -- serialized FIFO on the sync
HWDGE ring (parallel queues let the head's completion semaphore straggle
behind other queues' packets in the SDMA round-robin), with only ttb
overlapping on a SWDGE queue.  10 dummy warmup matmuls on a zeroed tile
keep the PE busy from the barrier until real data lands, bridging the HAM
clock-gate window (the PE runs 1.2 GHz until ~3.4 us of sustained
activity; any idle gap before the real stream delays the un-throttle by a
whole extra window on the slowest core, so the warmup deliberately
overshoots the typical input receipt).

Each (u-tile, d) group runs LDWEIGHTS once then streams 4 x N<=512 matmuls
into two 2-bank PSUM tiles; DVE and ACT alternate evacuating them with the
fp32->bf16 cast fused into wide [128, ~1000] copies (fp32 PSUM reads run at
1 elem/cycle/lane, so wide copies amortize the ~120-170 cycle fixed cost;
per-v-tile two-engine copies measured slower end-to-end three times).
The group's [128, 2001] bf16 output region (512 KB) is flushed to HBM as
soon as its two copies land, round-robined over 4 SWDGE queues, so the
output DMA (~8.9 us/core line-rate-dense window at the ~358 GB/s HBM
roofline) overlaps compute and paces the kernel; the first and last groups
flush per half-region to open the window earlier and shorten the
completion tail.  Flushes always cover all 128 partitions: partial-
partition DMAs were measured to unbalance the SDMA engine split ~3x.

Sharding: data-parallel over u. Each core computes a [251, 2001, 3] slab,
padded to 2x128 u-rows on device; the host drops the padding and
interleaves d.
"""

import numpy as np

N_CTRL = 64
N_EVAL = 2001
N_CORES = 8
NU_SHARD = 251   # ceil(2001 / 8); 8 * 251 = 2008 (last 7 rows are zero padding)
NU_PAD = 256     # per-core u padded to 2 full 128-wide PE column tiles
V_TILE = 512
V_HALF = 1024    # psum/copy/flush half-split of the v axis

_CACHE = {}


def _clamped_knots(p, n_ctrl, dtype=np.float64):
    n_internal = n_ctrl - p - 1
    internal = np.linspace(0.0, 1.0, n_internal + 2, dtype=dtype)[1:-1]
    return np.concatenate(
        [np.zeros(p + 1, dtype), internal, np.ones(p + 1, dtype)]
    )


def _dense_basis(params, p, n_ctrl):
    """Dense basis matrix A [len(params), n_ctrl], float64, with
    A[k, span-p+a] = B[k, a] (Cox-de-Boor, NURBS book A2.2)."""
    knots = _clamped_knots(p, n_ctrl)
    u = np.asarray(params, np.float64)
    spans = np.clip(np.searchsorted(knots, u, side="right") - 1, p, n_ctrl - 1)
    Ns = [np.ones_like(u)]
    left = {}
    right = {}
    for j in range(1, p + 1):
        left[j] = u - knots[spans + 1 - j]
        right[j] = knots[spans + j] - u
        saved = np.zeros_like(u)
        new = []
        for r in range(j):
            temp = Ns[r] / (right[r + 1] + left[j - r])
            new.append(saved + right[r + 1] * temp)
            saved = left[j - r] * temp
        new.append(saved)
        Ns = new
    B = np.stack(Ns, axis=-1)  # [N, p+1]
    A = np.zeros((len(u), n_ctrl), np.float64)
    rows = np.arange(len(u))[:, None]
    cols = spans[:, None] - p + np.arange(p + 1)[None, :]
    A[rows, cols] = B
    return A


def _pad_k128(a_bf16):
    """[64, N] bf16 -> [128, N] with zeros in partitions 64-127."""
    return np.ascontiguousarray(
        np.concatenate([a_bf16, np.zeros_like(a_bf16)], axis=0)
    )


# input tiles: name -> columns.  head = first weight group (d=0, g=0) plus
# the first Av.T v-tile fused into one tile so a single DMA receipt gates
# both the first LDWEIGHTS and the first matmul; ttb = the other five
# weight groups; avt2/3 = remaining Av.T columns in first-use order.
IN_COLS = {
    "head": 128 + V_TILE,
    "ttb": 3 * NU_PAD - 128,
    "avt2": V_TILE,
    "avt3": N_EVAL - V_HALF,
}


def _build_device():
    if "nc" in _CACHE:
        return _CACHE["nc"]

    import concourse.mybir as mybir
    import concourse.tile as tile
    from concourse import bacc

    f32 = mybir.dt.float32
    bf16 = mybir.dt.bfloat16
    nc = bacc.Bacc(
        "TRN2", target_bir_lowering=False, debug=False, num_devices=N_CORES,
        num_swdge_queues=4,
    )
    # inputs ship as [128, cols] with host-side zeros in rows 64-127: a
    # 64-partition DMA would engage only the 8 even SDMA engines (engine <->
    # partition mapping), measured slower and jitterier than moving 2x the
    # bytes through all 16
    ins = {
        name: nc.dram_tensor(name, [128, cols], bf16,
                             kind="ExternalInput").ap()
        for name, cols in IN_COLS.items()
    }
    # out col = g*6003 + d*2001 + v for u-tile g in {0, 1}
    out_h = nc.dram_tensor(
        "out", [128, 2 * 3 * N_EVAL], bf16, kind="ExternalOutput"
    ).ap()

    # (v0, width, avt tile, col offset within it) in stream order
    VT = [
        (0, V_TILE, "head", 128),
        (V_TILE, V_TILE, "avt2", 0),
        (V_HALF, V_TILE, "avt3", 0),
        (V_HALF + V_TILE, N_EVAL - V_HALF - V_TILE, "avt3", V_TILE),
    ]

    with tile.TileContext(nc) as tc:
        with (
            tc.tile_pool(name="consts", bufs=1) as consts,
            tc.tile_pool(name="ps", bufs=4, space="PSUM") as psp,
            tc.tile_pool(name="obuf", bufs=1) as obuf,
        ):
            sb = {
                name: consts.tile([128, cols], bf16, tag=name, name=name)
                for name, cols in IN_COLS.items()
            }
            # head/avt2/avt3 ride the sync HWDGE ring FIFO, first-use first:
            # serialized packets mean the head's completion semaphore can't
            # straggle behind other queues in the SDMA round-robin (the
            # interleaved 4-queue layout measured 10.3-11.6 us receipt
            # jitter on the slowest core).  ttb overlaps on a SWDGE queue.
            for name in ("head", "avt2", "avt3"):
                nc.sync.dma_start(out=sb[name], in_=ins[name])
            dma = nc.gpsimd.dma_start(out=sb["ttb"], in_=ins["ttb"])
            dma.ins.queue = "qPoolDynamic1"
            warm = consts.tile([128, V_TILE], bf16, tag="warm", name="warm")
            nc.vector.memset(warm, 0.0)

            # PE warmup: bridge barrier-to-data-landing so the HAM clock
            # gate sees sustained activity (cold PE = 1.2 GHz for ~3.4 us).
            # N=512 dummies measured the most reliable un-throttle (N=256
            # ones ran the same span but HAM fired ~4 us later).  Overshoot
            # the typical input receipt: an idle gap before the real stream
            # delays the un-throttle by a whole extra ~3.4 us window on the
            # slowest core, which is what the max-core exec time pays.
            for _ in range(10):
                wps = psp.tile([128, V_HALF], f32, tag="ps")
                nc.tensor.matmul(
                    wps[:, :V_TILE], warm[:, :128], warm,
                    start=True, stop=True,
                )

            ob = {
                g: obuf.tile([128, 3 * N_EVAL], bf16, tag=f"ob{g}",
                             name=f"ob{g}")
                for g in range(2)
            }

            prev_mm = None
            n_out = 0
            gi = 0
            for d in range(3):
                for g in range(2):
                    col = d * NU_PAD + g * 128
                    if col == 0:
                        w = sb["head"][:, 0:128]
                    else:
                        w = sb["ttb"][:, col - 128:col]
                    ldw = nc.tensor.ldweights(w)
                    if prev_mm is not None:
                        tile.add_dep_helper(
                            ldw.ins, prev_mm.ins, sync=False,
                            reason="weight group order",
                        )
                    # the first group flushes per half-region so the output
                    # DMA window (the line-rate-dense ~9 us that paces the
                    # kernel) opens as early as possible; the final group
                    # flushes per half to shorten the end-of-kernel
                    # copy+transfer+receipt tail
                    split = gi in (0, 5)
                    obase = g * 3 * N_EVAL + d * N_EVAL
                    for hi, (h0, hw) in enumerate(((0, V_HALF),
                                                   (V_HALF, N_EVAL - V_HALF))):
                        ps = psp.tile([128, V_HALF], f32, tag="ps")
                        for v0, vw, av, c0 in VT[2 * hi:2 * hi + 2]:
                            mm = nc.tensor.matmul(
                                ps[:, v0 - h0:v0 - h0 + vw], w,
                                sb[av][:, c0:c0 + vw],
                                start=True, stop=True,
                            )
                            mm.ins.ldweights = False
                            tile.add_dep_helper(
                                mm.ins, ldw.ins, sync=False,
                                reason="matmul after its ldweights",
                            )
                            prev_mm = mm
                        osl = slice(d * N_EVAL + h0, d * N_EVAL + h0 + hw)
                        # one wide copy per half; engines alternate and
                        # disjoint ranges run concurrently (per-v-tile
                        # two-engine copies were tried three times and
                        # always measured slower end-to-end)
                        if (hi == 0) == (gi % 2 == 0):
                            nc.vector.tensor_copy(ob[g][:, osl], ps[:, :hw])
                        else:
                            nc.scalar.copy(ob[g][:, osl], ps[:, :hw])
                        if split:
                            dma = nc.gpsimd.dma_start(
                                out=out_h[:, obase + h0:obase + h0 + hw],
                                in_=ob[g][:, osl],
                            )
                            dma.ins.queue = f"qPoolDynamic{n_out % 4 or ''}"
                            n_out += 1
                    if not split:
                        # flush this (u-tile, d) region once its copies land
                        osl = slice(d * N_EVAL, (d + 1) * N_EVAL)
                        dma = nc.gpsimd.dma_start(
                            out=out_h[:, obase:obase + N_EVAL],
                            in_=ob[g][:, osl],
                        )
                        dma.ins.queue = f"qPoolDynamic{n_out % 4 or ''}"
                        n_out += 1
                    gi += 1
    nc.compile()
    _CACHE["nc"] = nc
    return nc


def kernel(control_points, params_u, params_v, degree):
    import ml_dtypes
    from concourse.bass_utils import run_bass_kernel_spmd

    p = int(np.asarray(degree))
    cp = np.asarray(control_points, np.float32)
    pu = np.asarray(params_u, np.float32)
    pv = np.asarray(params_v, np.float32)
    assert cp.shape == (N_CTRL, N_CTRL, 3), cp.shape
    assert pu.shape == (N_EVAL,) and pv.shape == (N_EVAL,), (pu.shape, pv.shape)

    Au = np.zeros((N_CORES * NU_SHARD, N_CTRL), np.float64)
    Au[:N_EVAL] = _dense_basis(pu, p, N_CTRL)
    Av = _dense_basis(pv, p, N_CTRL)

    # host stage 1 (0.3% of the FLOPs): T[j, d, u] = sum_i P[i,j,d] Au[u,i]
    T = (cp.astype(np.float64).transpose(1, 2, 0).reshape(3 * N_CTRL, N_CTRL)
         @ Au.T).reshape(N_CTRL, 3, N_CORES * NU_SHARD)

    avt = Av.T.astype(np.float32).astype(ml_dtypes.bfloat16)
    avs = {
        "avt2": _pad_k128(np.ascontiguousarray(avt[:, V_TILE:V_HALF])),
        "avt3": _pad_k128(np.ascontiguousarray(avt[:, V_HALF:])),
    }

    nc = _build_device()
    in_maps = []
    for c in range(N_CORES):
        ttc = np.zeros((N_CTRL, 3, NU_PAD), np.float32)
        ttc[:, :, :NU_SHARD] = T[:, :, c * NU_SHARD:(c + 1) * NU_SHARD]
        tt = ttc.reshape(N_CTRL, 3 * NU_PAD).astype(ml_dtypes.bfloat16)
        in_maps.append({
            "head": _pad_k128(np.ascontiguousarray(
                np.concatenate([tt[:, :128], avt[:, :V_TILE]], axis=1)
            )),
            "ttb": _pad_k128(np.ascontiguousarray(tt[:, 128:])),
            **avs,
        })

    res = run_bass_kernel_spmd(
        nc,
        in_maps,
        core_ids=list(range(N_CORES)),
        trace=_CACHE.get("trace", False),
        **_CACHE.get("run_kwargs", {}),
    )
    _CACHE["last_result"] = res
    # out col = g*6003 + d*2001 + v; u-tile g=1 holds rows 128..250
    full = np.empty((N_CORES * NU_SHARD, 3, N_EVAL), np.float32)
    for c, r in enumerate(res.results):
        o = np.asarray(r["out"]).astype(np.float32)
        o = o.reshape(128, 2, 3, N_EVAL)  # cols are [g][d][v] row-major
        full[c * NU_SHARD:c * NU_SHARD + 128] = o[:, 0]
        full[c * NU_SHARD + 128:(c + 1) * NU_SHARD] = o[:NU_SHARD - 128, 1]
    return np.ascontiguousarray(full[:N_EVAL].transpose(0, 2, 1))
